# revision 1
# baseline (speedup 1.0000x reference)
"""Trainium2 kernel for nn_DigitConvolutionalModel (dense_cnn).

Model: x[B,784] -> 3x3 valid conv (single channel) -> flatten[676]
       -> Linear(676,200) + ReLU -> Linear(200,10).

The conv is linear, so it is folded into the first Linear on the host:
  flat = x @ C  (C [784,676] sparse conv matrix)
  h1   = relu(flat @ W1.T + b1) = relu(x @ (C @ W1.T) + b1)
so the device computes a plain 784 -> 200 -> 10 MLP. Pure data
parallelism: batch 32768 is split into 8 shards of 4096, one per core;
weights are replicated. Each core receives x pre-transposed ([784,4096],
pixel on the partition/contraction axis) so both matmuls need no
on-device transpose:
  FC1: h1T[200,b] = W1eff[784,200].T @ xT[784,b]   (lhsT = W1eff)
  FC2: outT[10,b] = W2T[200,10].T  @ h1T[200,b]    (lhsT = W2.T)
"""

import os
import numpy as np
from contextlib import ExitStack

import concourse.bass as bass
import concourse.bacc as bacc
import concourse.mybir as mybir
import concourse.tile as tile
from concourse.bass_utils import run_bass_kernel_spmd

import ml_dtypes

N_CORES = 8
B = 32768
BS = B // N_CORES          # 4096 rows per core
IMG = 28
KSZ = 3
OUTW = IMG - KSZ + 1       # 26
NPIX = IMG * IMG           # 784
HID = 200
NCLS = 10

P = 128                    # SBUF partitions
LO_C = 64                  # partition chunk per HWDGE ring
FD = 512                   # matmul free dim (ISA max moving elements; 1 PSUM bank)
NK = 7                     # contraction tiles over 784 = 6*128 + 16
KT = [P] * 6 + [NPIX - 6 * P]
MT = [P, HID - P]          # hid output tiles: 128 + 72
NHALF = 2                  # batch halves per core (PSUM: 2m x 4n = 8 banks)
HB = BS // NHALF           # 2048
NT = HB // FD              # 4 n-tiles of 512 per half

_cache: dict = {}


def _ensure_axon_hooks():
    """Provide antenv.axon_hooks if the image lacks it.

    bass_utils' trace path does `from antenv.axon_hooks import
    get_axon_ntff_profile_hook`; on images without that module the import
    crashes instead of degrading. Register a minimal equivalent that drives
    NTFF profiling via the documented C ABI of the loaded axon PJRT plugin
    (axon_start_nrt_profile / axon_stop_nrt_profile), or returns None so
    bass_utils skips tracing gracefully.
    """
    try:
        import antenv.axon_hooks  # noqa: F401

        return
    except ImportError:
        pass
    import sys
    import types
    import ctypes
    import contextlib

    try:
        import antenv
    except ImportError:
        antenv = types.ModuleType("antenv")
        sys.modules["antenv"] = antenv

    mod = types.ModuleType("antenv.axon_hooks")
    state = {"hook": None, "built": False}

    def _build():
        so_path = None
        try:
            with open("/proc/self/maps") as f:
                for line in f:
                    if "libaxon_pjrt.so" in line:
                        so_path = line.split()[-1]
                        break
        except OSError:
            return None
        if so_path is None:
            return None
        lib = ctypes.CDLL(so_path)
        if not hasattr(lib, "axon_start_nrt_profile"):
            return None
        lib.axon_start_nrt_profile.argtypes = [
            ctypes.POINTER(ctypes.c_int64),
            ctypes.c_size_t,
        ]
        lib.axon_start_nrt_profile.restype = ctypes.c_int64
        lib.axon_stop_nrt_profile.argtypes = [ctypes.c_char_p]
        lib.axon_stop_nrt_profile.restype = ctypes.c_int64

        @contextlib.contextmanager
        def _hook(output_dir, device_ids):
            import jax

            jax.devices()
            if device_ids:
                ids = (ctypes.c_int64 * len(device_ids))(*device_ids)
                rc = lib.axon_start_nrt_profile(ids, len(device_ids))
            else:
                rc = lib.axon_start_nrt_profile(None, 0)
            if rc != 0:
                raise RuntimeError(f"axon_start_nrt_profile rc={rc}")
            try:
                yield
            finally:
                n = lib.axon_stop_nrt_profile(str(output_dir).encode())
                if n <= 0:
                    print(f"ntff profile: rc={n} (no profile written)")

        return _hook

    def get_axon_ntff_profile_hook():
        if not state["built"]:
            state["hook"] = _build()
            state["built"] = True
        return state["hook"]

    def set_axon_ntff_profile_hook(hook):
        state["hook"] = hook
        state["built"] = True

    mod.get_axon_ntff_profile_hook = get_axon_ntff_profile_hook
    mod.set_axon_ntff_profile_hook = set_axon_ntff_profile_hook
    sys.modules["antenv.axon_hooks"] = mod
    antenv.axon_hooks = mod


def _dtypes():
    if os.environ.get("KERNEL_FP32"):
        return mybir.dt.float32, np.float32
    return mybir.dt.bfloat16, ml_dtypes.bfloat16


def _build_nc():
    mm_dt, _ = _dtypes()
    f32 = mybir.dt.float32
    # Bacc (not plain Bass): its compile() pass splits multi-sem waits into
    # standalone EventSemaphore instructions — the TPB ISA allows only one
    # embedded wait per instruction.
    nc = bacc.Bacc(
        "TRN2",
        target_bir_lowering=False,
        debug=False,
        num_devices=N_CORES,
    )

    xT = nc.dram_tensor("xT", [NPIX, BS], mm_dt, kind="ExternalInput")
    w1 = nc.dram_tensor("w1t", [P, NK * HID], mm_dt, kind="ExternalInput")
    w2 = nc.dram_tensor("w2t", [P, 2 * NCLS], mm_dt, kind="ExternalInput")
    b1 = nc.dram_tensor("b1t", [P, 2], f32, kind="ExternalInput")
    b2 = nc.dram_tensor("b2t", [NCLS, 1], f32, kind="ExternalInput")
    outT = nc.dram_tensor("outT", [NCLS, BS], f32, kind="ExternalOutput")

    with ExitStack() as ctx:
        tc = ctx.enter_context(tile.TileContext(nc))
        const = ctx.enter_context(tc.tile_pool(name="const", bufs=1))
        xp = ctx.enter_context(tc.tile_pool(name="xp", bufs=NHALF * NK))
        h1p = ctx.enter_context(tc.tile_pool(name="h1p", bufs=2))
        op = ctx.enter_context(tc.tile_pool(name="op", bufs=NHALF * NT))
        pp = ctx.enter_context(tc.tile_pool(name="pp", bufs=8, space="PSUM"))

        w1s = const.tile([P, NK * HID], mm_dt)
        w2s = const.tile([P, 2 * NCLS], mm_dt)
        b1s = const.tile([P, 2], f32)
        b2s = const.tile([NCLS, 1], f32)
        nc.sync.dma_start(w1s[:], w1[:])
        nc.sync.dma_start(w2s[:], w2[:])
        nc.sync.dma_start(b1s[:], b1[:])
        nc.sync.dma_start(b2s[:], b2[:])

        h1tiles = []
        for h in range(NHALF):
            c0 = h * HB
            ps = [
                [
                    pp.tile([MT[m], FD], f32, tag="bank", name=f"ps_{h}_{m}_{n}")
                    for n in range(NT)
                ]
                for m in range(2)
            ]
            for k in range(NK):
                kt = KT[k]
                xt = xp.tile([P, HB], mm_dt, tag="xt", name=f"xt_{h}_{k}")
                nc.sync.dma_start(xt[:kt, :], xT[k * P : k * P + kt, c0 : c0 + HB])
                for m in range(2):
                    lhsT = w1s[0:kt, k * HID + m * P : k * HID + m * P + MT[m]]
                    for n in range(NT):
                        nc.tensor.matmul(
                            ps[m][n][:],
                            lhsT,
                            xt[0:kt, n * FD : (n + 1) * FD],
                            start=(k == 0),
                            stop=(k == NK - 1),
                        )
            h1 = [
                h1p.tile([MT[0], HB], mm_dt, tag="h1a", name=f"h1a_{h}"),
                h1p.tile([MT[1], HB], mm_dt, tag="h1b", name=f"h1b_{h}"),
            ]
            # Drains split across ACT (m0, relu via LUT with bias) and DVE
            # (m1, add-bias then max-0) so the banks free twice as fast.
            for n in range(NT):
                nc.scalar.activation(
                    h1[0][:, n * FD : (n + 1) * FD],
                    ps[0][n][:],
                    mybir.ActivationFunctionType.Relu,
                    bias=b1s[0 : MT[0], 0:1],
                )
            for n in range(NT):
                nc.vector.tensor_scalar(
                    h1[1][:, n * FD : (n + 1) * FD],
                    ps[1][n][:],
                    b1s[0 : MT[1], 1:2],
                    0.0,
                    mybir.AluOpType.add,
                    mybir.AluOpType.max,
                )
            h1tiles.append(h1)

        # FC2 emitted after all FC1 matmuls: PE stays dense through FC1,
        # FC2 runs at the tail when h1 has long been drained.
        for h in range(NHALF):
            c0 = h * HB
            h1 = h1tiles[h]
            for n in range(NT):
                ps2 = pp.tile([NCLS, FD], f32, tag="bank", name=f"ps2_{h}_{n}")
                for m in range(2):
                    nc.tensor.matmul(
                        ps2[:],
                        w2s[0 : MT[m], m * NCLS : (m + 1) * NCLS],
                        h1[m][:, n * FD : (n + 1) * FD],
                        start=(m == 0),
                        stop=(m == 1),
                    )
                ot = op.tile([NCLS, FD], f32, tag="ot", name=f"ot_{h}_{n}")
                nc.vector.tensor_scalar_add(ot[:], ps2[:], b2s[:])
                nc.gpsimd.dma_start(
                    outT[:, c0 + n * FD : c0 + (n + 1) * FD], ot[:]
                )

    nc.compile()
    nc.finalize()
    return nc


def _build_nc_raw():
    """Hand-scheduled version (no TileContext): explicit per-engine streams
    and semaphores. Avoids Tile's prologue/epilogue barriers (~13us fixed)."""
    mm_dt, _ = _dtypes()
    f32 = mybir.dt.float32
    nc = bacc.Bacc(
        "TRN2",
        target_bir_lowering=False,
        debug=False,
        num_devices=N_CORES,
    )

    xT = nc.dram_tensor("xT", [NPIX, BS], mm_dt, kind="ExternalInput")
    w1 = nc.dram_tensor("w1t", [P, NK * HID], mm_dt, kind="ExternalInput")
    w2 = nc.dram_tensor("w2t", [P, 2 * NCLS], mm_dt, kind="ExternalInput")
    b1 = nc.dram_tensor("b1t", [P, 2], f32, kind="ExternalInput")
    b2 = nc.dram_tensor("b2r", [P, 1], f32, kind="ExternalInput")
    outT = nc.dram_tensor("outT", [NCLS, BS], f32, kind="ExternalOutput")

    NB = NHALF * NT  # output column blocks of FD
    NGRP = NB // NT  # FC2 col-packed groups

    # SBUF
    xts = [nc.alloc_sbuf_tensor(f"xt{k}", [P, BS], mm_dt).ap() for k in range(NK)]
    w1s = nc.alloc_sbuf_tensor("w1s", [P, NK * HID], mm_dt).ap()
    w2s = nc.alloc_sbuf_tensor("w2s", [P, 2 * NCLS], mm_dt).ap()
    b1s = nc.alloc_sbuf_tensor("b1s", [P, 2], f32).ap()
    b2s = nc.alloc_sbuf_tensor("b2s", [P, 1], f32).ap()
    h1a = nc.alloc_sbuf_tensor("h1a", [MT[0], BS], mm_dt).ap()
    h1b = nc.alloc_sbuf_tensor("h1b", [MT[1], BS], mm_dt).ap()
    ot = nc.alloc_sbuf_tensor("ot", [NCLS, BS], f32).ap()
    # zeroed operand for PE warm-up matmuls (results are discarded —
    # the first real matmul into each bank uses start=True)
    warm = nc.alloc_sbuf_tensor("warm", [P, 256], mm_dt).ap()

    # PSUM: 2m x NT tensors covering all 8 banks; FC2 reuses them.
    ps = [
        [nc.alloc_psum_tensor(f"ps_{m}_{n}", [MT[m], FD], f32).ap() for n in range(NT)]
        for m in range(2)
    ]
    ps_flat = [ps[0][n] for n in range(NT)] + [ps[1][n] for n in range(NT)]

    if True:
        # One sem per transfer (completions across a queue are unordered).
        # alloc_semaphore (not the ctx-manager nc.semaphore) — the ctx exit
        # emits a per-sem clear instruction; we do one range-clear instead.
        s_x = [
            [nc.alloc_semaphore(f"s_x_{h}_{k}") for k in range(NK)]
            for h in range(NHALF)
        ]
        s_x0b = nc.alloc_semaphore("s_x0b")
        s_warm = nc.alloc_semaphore("s_warm")
        s_w1a = nc.alloc_semaphore("s_w1a")
        s_w1a2 = nc.alloc_semaphore("s_w1a2")
        s_b1 = nc.alloc_semaphore("s_b1")
        s_b2 = nc.alloc_semaphore("s_b2")
        s_w2 = nc.alloc_semaphore("s_w2")
        s_mm = nc.alloc_semaphore("s_mm")
        s_da = nc.alloc_semaphore("s_da")
        s_dv = nc.alloc_semaphore("s_dv")
        s_f2a = nc.alloc_semaphore("s_f2a")
        s_f2b = nc.alloc_semaphore("s_f2b")
        s_out = nc.alloc_semaphore("s_out")
        s_out2 = nc.alloc_semaphore("s_out2")
        all_sems = (
            [s for h in s_x for s in h]
            + [s_x0b, s_warm]
            + [s_w1a, s_w1a2, s_b1, s_b2, s_w2, s_mm, s_da, s_dv,
               s_f2a, s_f2b, s_out, s_out2]
        )

        # closer index (1-based s_mm value) of the last matmul into bank (h,m,n)
        def closer(h, m, n):
            return h * 2 * NT + m * NT + n + 1

        with nc.Block(no_gpsimd_drain=True) as block:

            @block.gpsimd
            def _(gpsimd):
                gpsimd.memset(warm[:], 0.0).then_inc(s_warm, 1)

            @block.sync
            def _(sync):
                # all x on this ring; every piece spans 128 partitions so the
                # SDMA engines stay port-aligned (full ring rate). k=0/1
                # weight slice first so the first matmuls can start early.
                for h in range(NHALF):
                    c0 = h * HB
                    for k in range(NK):
                        kt = KT[k]
                        if h == 0 and k == 0:
                            # first piece split in two: compute starts earlier
                            sync.dma_start(
                                xts[0][:, 0 : HB // 2], xT[0:P, 0 : HB // 2]
                            ).then_inc(s_x[0][0], 16)
                            sync.dma_start(
                                xts[0][:, HB // 2 : HB], xT[0:P, HB // 2 : HB]
                            ).then_inc(s_x0b, 16)
                            sync.dma_start(
                                w1s[:, HID:], w1[:, HID:]
                            ).then_inc(s_w1a2, 16)
                            continue
                        sync.dma_start(
                            xts[k][0:kt, c0 : c0 + HB],
                            xT[k * P : k * P + kt, c0 : c0 + HB],
                        ).then_inc(s_x[h][k], 16)
                # quarters 2,3 of the output on this ring
                QW = BS // 4
                for q in (2, 3):
                    sync.wait_ge(s_f2a, q + 1)
                    sync.wait_ge(s_f2b, q + 1)
                    sync.dma_start(
                        outT[:, q * QW : (q + 1) * QW], ot[0:NCLS, q * QW : (q + 1) * QW]
                    ).then_inc(s_out, 16)
                sync.wait_ge(s_out, 32)

            @block.scalar
            def _(scalar):
                scalar.dma_start(w1s[:, 0:HID], w1[:, 0:HID]).then_inc(
                    s_w1a, 16
                )
                scalar.dma_start(b1s[:], b1[:]).then_inc(s_b1, 16)
                scalar.dma_start(b2s[:], b2[:]).then_inc(s_b2, 16)
                scalar.dma_start(w2s[:], w2[:]).then_inc(s_w2, 16)
                # FC1 m0 drains: relu + bias from PSUM -> h1a (bf16 cast)
                scalar.wait_ge(s_b1, 16)
                for h in range(NHALF):
                    for n in range(NT):
                        scalar.wait_ge(s_mm, closer(h, 0, n))
                        c = h * HB + n * FD
                        nc.scalar.activation(
                            h1a[:, c : c + FD],
                            ps[0][n][:],
                            mybir.ActivationFunctionType.Relu,
                            bias=b1s[0 : MT[0], 0:1],
                        ).then_inc(s_da, 1)
                # FC2 drains for odd blocks (ACT side — faster, takes the
                # last block), b2 bias + identity
                scalar.wait_ge(s_b2, 16)
                for b in range(1, NB, 2):
                    scalar.wait_ge(s_mm, 2 * 2 * NT + b + 1)
                    j = b % NT
                    nc.scalar.activation(
                        ot[:, b * FD : (b + 1) * FD],
                        ps_flat[b // NT][32 * j : 32 * j + NCLS, :],
                        mybir.ActivationFunctionType.Identity,
                        bias=b2s[0:NCLS, :],
                    ).then_inc(s_f2a, 1)
                # quarters 0,1 of the output on this ring
                QW = BS // 4
                for q in (0, 1):
                    scalar.wait_ge(s_f2a, q + 1)
                    scalar.wait_ge(s_f2b, q + 1)
                    scalar.dma_start(
                        outT[:, q * QW : (q + 1) * QW], ot[0:NCLS, q * QW : (q + 1) * QW]
                    ).then_inc(s_out2, 16)
                scalar.wait_ge(s_out2, 32)

            @block.tensor
            def _(tensor):
                # Warm-up: keep the PE active while the first x piece loads so
                # HAM un-throttles to 2.4 GHz before real matmuls start.
                tensor.wait_ge(s_warm, 1)
                for _ in range(32):
                    nc.tensor.matmul(
                        ps[0][0][0:P, 0:256], warm[:, 0:P], warm[:],
                        start=True, stop=True,
                    )
                tensor.wait_ge(s_w1a, 16)
                for h in range(NHALF):
                    for k in range(NK):
                        kt = KT[k]
                        if k == 1:
                            # remainder of w1 (k>=1 column blocks)
                            tensor.wait_ge(s_w1a2, 16)
                        tensor.wait_ge(s_x[h][k], 16)
                        if h == 0 and k == 0:
                            # n-tiles 0,1 (both m) run off the first column
                            # sub-piece; 2,3 wait for the second
                            for nn in range(NT):
                                if nn == 2:
                                    tensor.wait_ge(s_x0b, 16)
                                for m in range(2):
                                    nc.tensor.matmul(
                                        ps[m][nn][:],
                                        w1s[0:kt, m * P : m * P + MT[m]],
                                        xts[0][0:kt, nn * FD : (nn + 1) * FD],
                                        start=True,
                                        stop=False,
                                    )
                            continue
                        for m in range(2):
                            lhsT = w1s[0:kt, k * HID + m * P : k * HID + m * P + MT[m]]
                            for n in range(NT):
                                if h == 1 and k == 0:
                                    # bank reuse: wait for phase-0 drain (WAR)
                                    if m == 0:
                                        tensor.wait_ge(s_da, n + 1)
                                    else:
                                        tensor.wait_ge(s_dv, n + 1)
                                mm = nc.tensor.matmul(
                                    ps[m][n][:],
                                    lhsT,
                                    xts[k][0:kt, h * HB + n * FD : h * HB + (n + 1) * FD],
                                    start=(k == 0),
                                    stop=(k == NK - 1),
                                )
                                if k == NK - 1:
                                    mm.then_inc(s_mm, 1)
                # FC2 col-packed: group g of NT blocks shares one free
                # m0 bank; block j writes PE column-group j (concurrent MMs)
                tensor.wait_ge(s_w2, 16)
                for grp in range(NB // NT):
                    bank = ps_flat[grp]  # ps[0][grp]: free after phase-1 drain
                    tensor.wait_ge(s_da, NT + grp + 1)
                    for j in range(NT):
                        b = grp * NT + j
                        tensor.wait_ge(s_da, b + 1)
                        tensor.wait_ge(s_dv, b + 1)
                        c = b * FD
                        o = bank[32 * j : 32 * j + NCLS, :]
                        nc.tensor.matmul(
                            o,
                            w2s[0 : MT[0], 0:NCLS],
                            h1a[:, c : c + FD],
                            start=True,
                            stop=False,
                            tile_position=(0, 32 * j),
                            skip_group_check=True,
                        )
                        nc.tensor.matmul(
                            o,
                            w2s[0 : MT[1], NCLS : 2 * NCLS],
                            h1b[:, c : c + FD],
                            start=False,
                            stop=True,
                            tile_position=(0, 32 * j),
                            skip_group_check=True,
                        ).then_inc(s_mm, 1)

            @block.vector
            def _(vector):
                vector.wait_ge(s_b1, 16)
                vector.wait_ge(s_b2, 16)
                # FC1 m1 drains: (psum + b1) max 0 -> h1b (bf16 cast)
                for h in range(NHALF):
                    for n in range(NT):
                        vector.wait_ge(s_mm, closer(h, 1, n))
                        c = h * HB + n * FD
                        nc.vector.tensor_scalar(
                            h1b[:, c : c + FD],
                            ps[1][n][:],
                            b1s[0 : MT[1], 1:2],
                            0.0,
                            mybir.AluOpType.add,
                            mybir.AluOpType.max,
                        ).then_inc(s_dv, 1)
                # FC2 drains for even blocks (DVE side): psum + b2 -> ot
                vector.wait_ge(s_b2, 16)
                for b in range(0, NB, 2):
                    vector.wait_ge(s_mm, 2 * 2 * NT + b + 1)
                    j = b % NT
                    nc.vector.tensor_scalar_add(
                        ot[0:NCLS, b * FD : (b + 1) * FD],
                        ps_flat[b // NT][32 * j : 32 * j + NCLS, :],
                        b2s[0:NCLS, :],
                    ).then_inc(s_f2b, 1)

        # After the block-exit all-engine barrier every engine is synced;
        # reset sems so a re-execution of the NEFF starts clean.
        ids = sorted(s.num for s in all_sems)
        if ids == list(range(ids[0], ids[-1] + 1)):
            nc.gpsimd.sem_clear(range(ids[0], ids[-1] + 1))
        else:
            for s in all_sems:
                nc.gpsimd.sem_clear(s)

    nc.compile()
    nc.finalize()
    return nc


def _fold_weights(conv_w, W1):
    """W1eff[784,200] such that x @ W1eff == conv2d_valid(x, conv_w).flat @ W1.T"""
    W1r = W1.reshape(HID, OUTW, OUTW).transpose(1, 2, 0)  # [26,26,200]
    w1e = np.zeros((IMG, IMG, HID), np.float32)
    for di in range(KSZ):
        for dj in range(KSZ):
            w1e[di : di + OUTW, dj : dj + OUTW, :] += conv_w[di, dj] * W1r
    return w1e.reshape(NPIX, HID)


def _prepare_maps(x, conv_w, W1, b1, W2, b2, impl="raw"):
    _, np_dt = _dtypes()
    x = np.asarray(x, np.float32)
    conv_w = np.asarray(conv_w, np.float32)
    W1 = np.asarray(W1, np.float32)
    b1 = np.asarray(b1, np.float32)
    W2 = np.asarray(W2, np.float32)
    b2 = np.asarray(b2, np.float32)

    w1e = _fold_weights(conv_w, W1)
    w1t = np.zeros((P, NK * HID), np_dt)
    for k in range(NK):
        kt = KT[k]
        w1t[:kt, k * HID : (k + 1) * HID] = w1e[k * P : k * P + kt, :].astype(np_dt)
    W2T = W2.T  # [200, 10]
    w2t = np.zeros((P, 2 * NCLS), np_dt)
    w2t[: MT[0], 0:NCLS] = W2T[:P].astype(np_dt)
    w2t[: MT[1], NCLS : 2 * NCLS] = W2T[P:].astype(np_dt)
    b1t = np.zeros((P, 2), np.float32)
    b1t[: MT[0], 0] = b1[:P]
    b1t[: MT[1], 1] = b1[P:]
    b2t = b2.reshape(NCLS, 1)
    b2rv = np.zeros((P, 1), np.float32)
    for j in range(4):
        b2rv[32 * j : 32 * j + NCLS, 0] = b2

    xs = x.reshape(N_CORES, BS, NPIX)
    maps = []
    for i in range(N_CORES):
        xTi = xs[i].T.astype(np_dt)  # [784, 4096]
        m = {"w1t": w1t, "w2t": w2t, "b1t": b1t, "b2t": b2t, "b2r": b2rv}
        m["xT"] = xTi
        maps.append(m)
    return maps


def _run(inputs, trace=False):
    _ensure_axon_hooks()
    impl = os.environ.get("KERNEL_IMPL", "raw")
    # Build a fresh Bass module per call: re-executing an already-loaded
    # NEFF through this execution path wedges the device, so each call gets
    # its own executable (the NEFF compile cache keeps this cheap).
    nc = _build_nc_raw() if impl == "raw" else _build_nc()
    in_maps = _prepare_maps(**inputs, impl=impl)
    res = run_bass_kernel_spmd(nc, in_maps, list(range(N_CORES)), trace=trace)
    out = np.concatenate([r["outT"].T for r in res.results], axis=0)
    return out, res


def kernel(**inputs):
    out, _ = _run(inputs, trace=False)
    return out

